# revision 13
# baseline (speedup 1.0000x reference)
"""Dual cross-attention (AttentionA) Trainium2 kernel.

Sharding: 8 cores = 4 batches x 2 head-groups (8 heads each).
Per core (batch b, head-group g):
  xn=LN(x_b), xan=LN(xa_b); q,v from xn; ka,va from xan (group's 512 cols)
  per head: S = q ka^T (shared scores), softmax both directions,
  x_upd / xa_upd, partial out-projection with the group's Wout rows.
Host sums the two head-group partials per batch.

Matmuls run as float32r (full-rate fp32 mode, ~1.5e-4 rel err);
layernorm/softmax bookkeeping in fp32. All DMA via SWDGE (gpsimd) --
HWDGE (nc.sync) deadlocks under this runtime config.
"""

import numpy as np

B, N, D = 4, 1024, 1024
P = 128          # partitions
HG = 512         # head-group width per core (8 heads x 64)
HD = 64          # head dim
NT = N // P      # 8 n-tiles
DT = D // P      # 8 d-chunks
CT = HG // P     # 4 c-blocks (head pairs) per group
EPS = 1e-5

_cache = {}


def _build_program(debug=False):
    import concourse.bacc as bacc
    import concourse.mybir as mybir
    from concourse import tile, masks

    F32 = mybir.dt.float32
    F32R = mybir.dt.float32r
    BF16 = mybir.dt.bfloat16
    AF = mybir.ActivationFunctionType
    OP = mybir.AluOpType

    nc = bacc.Bacc("TRN2", target_bir_lowering=False, debug=False, num_devices=1)

    def inp(name, shape):
        return nc.dram_tensor(name, shape, F32, kind="ExternalInput").ap()

    def outp(name, shape):
        return nc.dram_tensor(name, shape, F32, kind="ExternalOutput").ap()

    xb_d = inp("xb", [N, D])
    xab_d = inp("xab", [N, D])
    lnw_d = inp("lnw", [D])
    lnb_d = inp("lnb", [D])
    wq_d = inp("wq", [D, HG])
    wk_d = inp("wk", [D, HG])
    wv_d = inp("wv", [D, HG])
    wo_d = inp("wo", [HG, D])
    ox_d = outp("ox", [N, D])
    oxa_d = outp("oxa", [N, D])
    rx_dram = nc.dram_tensor("rx_scratch", [16, N], F32).ap()
    dbg = {}
    if debug:
        for nm, shp in (("d_xnT", [D, N]), ("d_qT", [HG, N]), ("d_kaT", [HG, N]),
                        ("d_v", [NT * P, HG]), ("d_va", [NT * P, HG]),
                        ("d_e0", [P, N]), ("d_rx0", [P, NT]),
                        ("d_rows", [2, N]), ("d_rbx", [2, N]),
                        ("d_xu", [P, N]), ("d_xau", [P, N])):
            dbg[nm] = outp(nm, shp)

    DMA = nc.gpsimd.dma_start

    with tile.TileContext(nc) as tc:
        with (
            tc.tile_pool(name="persist", bufs=1) as pp,
            tc.tile_pool(name="slabs", bufs=1) as sp,
        ):
            # ---- constants ----
            ident = pp.tile([P, P], F32, tag="ident", name="ident")
            masks.make_identity(nc, ident[:])
            wcol = pp.tile([P, DT], F32, tag="wcol", name="wcol")
            bcolr = pp.tile([P, DT], F32R, tag="bcolr", name="bcolr")
            DMA(wcol[:], lnw_d.rearrange("(t p) -> p t", p=P))
            DMA(bcolr[:], lnb_d.rearrange("(t p) -> p t", p=P).bitcast(F32R))
            bcol = bcolr[:].bitcast(F32)

            # ---- persistent slabs (f32r: feed matmuls) ----
            qT = [sp.tile([P, N], F32R, tag=f"qT{t}", name=f"qT{t}") for t in range(CT)]
            kaT = [sp.tile([P, N], F32R, tag=f"kaT{t}", name=f"kaT{t}") for t in range(CT)]
            v_s = [sp.tile([P, HG], BF16, tag=f"v{i}", name=f"v{i}") for i in range(NT)]
            va_s = [sp.tile([P, HG], BF16, tag=f"va{i}", name=f"va{i}") for i in range(NT)]

            # psum helpers: sA/sB = 2-bank tiles, mA..mD = 1-bank tiles
            def ps_s(qpool, tag):
                return qpool.tile([P, 1024], F32, tag=tag, name="ps" + tag)

            def ps_m(qpool, tag):
                return qpool.tile([P, 512], F32, tag=tag, name="pm" + tag)

            with (
                tc.tile_pool(name="xnt", bufs=1) as xp,
                tc.tile_pool(name="work", bufs=1) as wp,
                tc.tile_pool(name="wstream", bufs=3) as wsp,
                tc.tile_pool(name="psum1", bufs=1, space="PSUM") as qq,
            ):
                xnT = [xp.tile([P, N], F32R, tag=f"xnT{j}", name=f"xnT{j}") for j in range(DT)]
                xanT = [xp.tile([P, N], F32R, tag=f"xanT{j}", name=f"xanT{j}") for j in range(DT)]

                # ---- LN + transpose (x then xa) ----
                for src_d, dstT in ((xb_d, xnT), (xab_d, xanT)):
                    xh = []
                    for i in range(NT):
                        xt = wp.tile([P, D], F32, tag=f"xt{i}", name=f"xt{i}")
                        DMA(xt[:], src_d[i * P:(i + 1) * P, :])
                        st = wp.tile([P, 12], F32, tag="bnst", name="bnst")
                        nc.vector.bn_stats(st[:, 0:6], xt[:, 0:512])
                        nc.vector.bn_stats(st[:, 6:12], xt[:, 512:1024])
                        ag = wp.tile([P, 2], F32, tag="bnag", name="bnag")
                        nc.vector.bn_aggr(ag[:], st[:])
                        veps = wp.tile([P, 1], F32, tag="veps", name="veps")
                        nc.vector.tensor_scalar(veps[:], ag[:, 1:2], float(EPS),
                                                None, op0=OP.add)
                        sq = wp.tile([P, 1], F32, tag="sq", name="sq")
                        nc.scalar.activation(sq[:], veps[:], AF.Sqrt)
                        rstd = wp.tile([P, 1], F32, tag="rstd", name="rstd")
                        nc.vector.reciprocal(rstd[:], sq[:])
                        # in-place: xt <- (xt - mean) * rstd
                        nc.vector.tensor_scalar(xt[:], xt[:], ag[:, 0:1], rstd[:],
                                                op0=OP.subtract, op1=OP.mult)
                        xh.append(xt)
                    for j in range(DT):
                        pt = ps_s(qq, "sA" if j % 2 == 0 else "sB")
                        for i in range(NT):
                            nc.tensor.transpose(pt[:, i * P:(i + 1) * P],
                                                xh[i][:, j * P:(j + 1) * P],
                                                ident[:])
                        for h2 in range(2):
                            sl = slice(h2 * 512, (h2 + 1) * 512)
                            nc.vector.scalar_tensor_tensor(
                                out=dstT[j][:, sl], in0=pt[:, sl],
                                scalar=wcol[:, j:j + 1],
                                in1=bcol[:, j:j + 1].to_broadcast((P, 512)),
                                op0=OP.mult, op1=OP.add)

                # ---- projections ----
                # qT/kaT: [c-block t, n] accum over d; regions 0..7
                for w_d, dst, srcT in ((wq_d, qT, xnT),
                                       (wk_d, kaT, xanT)):
                    grp = [ps_m(qq, "mA"), ps_m(qq, "mB"), ps_m(qq, "mC"),
                           ps_m(qq, "mD"), ps_s(qq, "sA"), ps_s(qq, "sB")]
                    regions = [grp[0][:], grp[1][:], grp[2][:], grp[3][:],
                               grp[4][:, 0:512], grp[4][:, 512:1024],
                               grp[5][:, 0:512], grp[5][:, 512:1024]]
                    for j in range(DT):
                        wt = wsp.tile([P, HG], F32R, tag="w", name="w")
                        DMA(wt[:], w_d[j * P:(j + 1) * P, :].bitcast(F32R))
                        for t in range(CT):
                            for nh in range(2):
                                nc.tensor.matmul(
                                    regions[t * 2 + nh],
                                    wt[:, t * P:(t + 1) * P],
                                    srcT[j][:, nh * 512:(nh + 1) * 512],
                                    start=(j == 0), stop=(j == DT - 1))
                    for t in range(CT):
                        for nh in range(2):
                            nc.vector.tensor_copy(
                                dst[t][:, nh * 512:(nh + 1) * 512],
                                regions[t * 2 + nh])
                # v/va: [n-block i, c] accum over d
                for w_d, dst, srcT in ((wv_d, v_s, xnT), (wv_d, va_s, xanT)):
                    grp = [ps_m(qq, "mA"), ps_m(qq, "mB"), ps_m(qq, "mC"),
                           ps_m(qq, "mD"), ps_s(qq, "sA"), ps_s(qq, "sB")]
                    regions = [grp[0][:], grp[1][:], grp[2][:], grp[3][:],
                               grp[4][:, 0:512], grp[4][:, 512:1024],
                               grp[5][:, 0:512], grp[5][:, 512:1024]]
                    for j in range(DT):
                        wt = wsp.tile([P, HG], F32R, tag="w", name="w")
                        DMA(wt[:], w_d[j * P:(j + 1) * P, :].bitcast(F32R))
                        for i in range(NT):
                            nc.tensor.matmul(
                                regions[i],
                                srcT[j][:, i * P:(i + 1) * P],
                                wt[:],
                                start=(j == 0), stop=(j == DT - 1))
                    for i in range(NT):
                        nc.vector.tensor_copy(dst[i][:], regions[i])
                if debug:
                    for j in range(DT):
                        DMA(dbg["d_xnT"][j * P:(j + 1) * P, :], xnT[j][:].bitcast(F32))

            # xnt/work/wstream/psum1 released here
            with (
                tc.tile_pool(name="head", bufs=1) as hp_,
                tc.tile_pool(name="expp", bufs=3) as ep,
                tc.tile_pool(name="psum2", bufs=1, space="PSUM") as q2,
            ):
                wo_s = []
                for cc in range(CT):
                    t = hp_.tile([P, D], F32R, tag=f"wo{cc}", name=f"wo{cc}")
                    DMA(t[:], wo_d[cc * P:(cc + 1) * P, :].bitcast(F32R))
                    wo_s.append(t)
                xupdT = [hp_.tile([P, N], F32R, tag=f"xu{t}", name=f"xu{t}") for t in range(CT)]
                xaupdT = [hp_.tile([P, N], F32R, tag=f"xau{t}", name=f"xau{t}") for t in range(CT)]

                def recip_rows(rcol0, rcol1, slot, tagp):
                    """[128, 8] per-head rowsum cols -> [128, N] recip bcast."""
                    rr0 = hp_.tile([P, NT], F32, tag="rr0", name="rr0")
                    rr1 = hp_.tile([P, NT], F32, tag="rr1", name="rr1")
                    nc.vector.reciprocal(rr0[:], rcol0[:])
                    nc.vector.reciprocal(rr1[:], rcol1[:])
                    pt = ps_s(q2, "sA")
                    nc.tensor.transpose(pt[0:NT, 0:P], rr0[:], ident[:])
                    nc.tensor.transpose(pt[0:NT, P:2 * P], rr1[:], ident[:])
                    rstage = hp_.tile([NT, 2 * P], F32, tag="rstage", name="rstage")
                    nc.vector.tensor_copy(rstage[:], pt[0:NT, 0:2 * P])
                    DMA(rx_dram[slot:slot + 2, :].rearrange(
                        "r (f p) -> f r p", p=P), rstage[:].rearrange(
                        "f (r p) -> f r p", p=P))
                    row0 = hp_.tile([1, N], F32, tag=f"{tagp}w0", name=f"{tagp}w0")
                    row1 = hp_.tile([1, N], F32, tag=f"{tagp}w1", name=f"{tagp}w1")
                    DMA(row0[:], rx_dram[slot, :].rearrange("n -> () n"))
                    DMA(row1[:], rx_dram[slot + 1, :].rearrange("n -> () n"))
                    rb0 = hp_.tile([P, N], F32, tag=f"{tagp}b0", name=f"{tagp}b0")
                    rb1 = hp_.tile([P, N], F32, tag=f"{tagp}b1", name=f"{tagp}b1")
                    nc.gpsimd.partition_broadcast(rb0[:], row0[:])
                    nc.gpsimd.partition_broadcast(rb1[:], row1[:])
                    return rb0, rb1

                if debug:
                    for t in range(CT):
                        DMA(dbg["d_qT"][t * P:(t + 1) * P, :], qT[t][:].bitcast(F32))
                        DMA(dbg["d_kaT"][t * P:(t + 1) * P, :], kaT[t][:].bitcast(F32))
                    dvst = hp_.tile([P, HG], F32, tag="dvst", name="dvst")
                    for i in range(NT):
                        nc.vector.tensor_copy(dvst[:], v_s[i][:])
                        DMA(dbg["d_v"][i * P:(i + 1) * P, :], dvst[:])
                        nc.vector.tensor_copy(dvst[:], va_s[i][:])
                        DMA(dbg["d_va"][i * P:(i + 1) * P, :], dvst[:])
                for hpi in range(CT):
                    h0c = slice((2 * hpi) * HD % HG, (2 * hpi) * HD % HG + HD)
                    h1c = slice((2 * hpi + 1) * HD % HG,
                                (2 * hpi + 1) * HD % HG + HD)
                    rx0 = hp_.tile([P, NT], F32, tag="rx0", name="rx0")
                    rx1 = hp_.tile([P, NT], F32, tag="rx1", name="rx1")
                    rxa0 = hp_.tile([P, NT], F32, tag="rxa0", name="rxa0")
                    rxa1 = hp_.tile([P, NT], F32, tag="rxa1", name="rxa1")

                    ps_xa0 = ps_m(q2, "mC")
                    ps_xa1 = ps_m(q2, "mD")
                    # E phase: S[i] = q ka^T; xa_upd accumulates over n
                    for i in range(NT):
                        sa = ps_s(q2, "sA")
                        sb = ps_s(q2, "sB")
                        for mh in range(2):
                            nc.tensor.matmul(
                                sa[:, mh * 512:(mh + 1) * 512],
                                qT[hpi][0:64, i * P:(i + 1) * P],
                                kaT[hpi][0:64, mh * 512:(mh + 1) * 512],
                                start=True, stop=True, tile_position=(0, 0))
                            nc.tensor.matmul(
                                sb[:, mh * 512:(mh + 1) * 512],
                                qT[hpi][64:128, i * P:(i + 1) * P],
                                kaT[hpi][64:128, mh * 512:(mh + 1) * 512],
                                start=True, stop=True, tile_position=(64, 0))
                        e0 = ep.tile([P, N], BF16, tag="E0", name="E0")
                        e1 = ep.tile([P, N], BF16, tag="E1", name="E1")
                        nc.scalar.activation(e0[:], sa[:], AF.Exp,
                                             accum_out=rx0[:, i:i + 1])
                        if debug and hpi == 0 and i == 0:
                            de = hp_.tile([P, N], F32, tag="de", name="de")
                            nc.vector.tensor_copy(de[:], e0[:])
                            DMA(dbg["d_e0"][:], de[:])
                        nc.scalar.activation(e1[:], sb[:], AF.Exp,
                                             accum_out=rx1[:, i:i + 1])
                        for mh, psxa in ((0, ps_xa0), (1, ps_xa1)):
                            nc.tensor.matmul(
                                psxa[0:64, :], v_s[i][:, h0c],
                                e0[:, mh * 512:(mh + 1) * 512],
                                start=(i == 0), stop=(i == NT - 1),
                                tile_position=(0, 0), skip_group_check=True)
                            nc.tensor.matmul(
                                psxa[64:128, :], v_s[i][:, h1c],
                                e1[:, mh * 512:(mh + 1) * 512],
                                start=(i == 0), stop=(i == NT - 1),
                                tile_position=(0, 64), skip_group_check=True)

                    if debug and hpi == 0:
                        DMA(dbg["d_rx0"][:], rx0[:])
                    rbx0, rbx1 = recip_rows(rx0, rx1, 4 * hpi, "rx")
                    if debug and hpi == 0:
                        DMA(dbg["d_rbx"][0:1, :], rbx0[0:1, :])
                        DMA(dbg["d_rbx"][1:2, :], rbx1[127:128, :])

                    ps_x0 = ps_m(q2, "mA")
                    ps_x1 = ps_m(q2, "mB")
                    # ET phase: S^T[j]; x_upd accumulates over m
                    for j in range(NT):
                        sa = ps_s(q2, "sA")
                        sb = ps_s(q2, "sB")
                        for nh in range(2):
                            nc.tensor.matmul(
                                sa[:, nh * 512:(nh + 1) * 512],
                                kaT[hpi][0:64, j * P:(j + 1) * P],
                                qT[hpi][0:64, nh * 512:(nh + 1) * 512],
                                start=True, stop=True, tile_position=(0, 0))
                            nc.tensor.matmul(
                                sb[:, nh * 512:(nh + 1) * 512],
                                kaT[hpi][64:128, j * P:(j + 1) * P],
                                qT[hpi][64:128, nh * 512:(nh + 1) * 512],
                                start=True, stop=True, tile_position=(64, 0))
                        et0 = ep.tile([P, N], BF16, tag="E0", name="E0")
                        et1 = ep.tile([P, N], BF16, tag="E1", name="E1")
                        nc.scalar.activation(et0[:], sa[:], AF.Exp,
                                             accum_out=rxa0[:, j:j + 1])
                        nc.scalar.activation(et1[:], sb[:], AF.Exp,
                                             accum_out=rxa1[:, j:j + 1])
                        for nh, psx in ((0, ps_x0), (1, ps_x1)):
                            nc.tensor.matmul(
                                psx[0:64, :], va_s[j][:, h0c],
                                et0[:, nh * 512:(nh + 1) * 512],
                                start=(j == 0), stop=(j == NT - 1),
                                tile_position=(0, 0), skip_group_check=True)
                            nc.tensor.matmul(
                                psx[64:128, :], va_s[j][:, h1c],
                                et1[:, nh * 512:(nh + 1) * 512],
                                start=(j == 0), stop=(j == NT - 1),
                                tile_position=(0, 64), skip_group_check=True)

                    rbxa0, rbxa1 = recip_rows(rxa0, rxa1, 4 * hpi + 2, "rxa")

                    for nh, psx in ((0, ps_x0), (1, ps_x1)):
                        sl = slice(nh * 512, (nh + 1) * 512)
                        nc.vector.tensor_tensor(out=xupdT[hpi][0:64, sl],
                                                in0=psx[0:64, :],
                                                in1=rbx0[0:64, sl], op=OP.mult)
                        nc.vector.tensor_tensor(out=xupdT[hpi][64:128, sl],
                                                in0=psx[64:128, :],
                                                in1=rbx1[64:128, sl], op=OP.mult)
                    for mh, psxa in ((0, ps_xa0), (1, ps_xa1)):
                        sl = slice(mh * 512, (mh + 1) * 512)
                        nc.vector.tensor_tensor(out=xaupdT[hpi][0:64, sl],
                                                in0=psxa[0:64, :],
                                                in1=rbxa0[0:64, sl], op=OP.mult)
                        nc.vector.tensor_tensor(out=xaupdT[hpi][64:128, sl],
                                                in0=psxa[64:128, :],
                                                in1=rbxa1[64:128, sl], op=OP.mult)

                if debug:
                    dxu = hp_.tile([P, N], F32, tag="dxu", name="dxu")
                    nc.vector.tensor_copy(dxu[:], xupdT[0][:].bitcast(F32))
                    DMA(dbg["d_xu"][:], dxu[:])
                    nc.vector.tensor_copy(dxu[:], xaupdT[0][:].bitcast(F32))
                    DMA(dbg["d_xau"][:], dxu[:])
                # ---- out-projection ----
                mtags = ["mA", "mB", "mC", "mD"]
                k = 0
                for slab, o_d in ((xupdT, ox_d), (xaupdT, oxa_d)):
                    for i in range(NT):
                        ob = hp_.tile([P, 1024], F32, tag="ob", name="ob",
                                      bufs=3)
                        for nh in range(2):
                            g = ps_m(q2, mtags[k % 4]); k += 1
                            for cc in range(CT):
                                nc.tensor.matmul(
                                    g[:], slab[cc][:, i * P:(i + 1) * P],
                                    wo_s[cc][:, nh * 512:(nh + 1) * 512],
                                    start=(cc == 0), stop=(cc == CT - 1))
                            nc.vector.tensor_copy(
                                ob[:, nh * 512:(nh + 1) * 512], g[:])
                        DMA(o_d[i * P:(i + 1) * P, :], ob[:])

    nc.finalize()
    return nc


def _get_program(debug=False):
    key = "ncdbg" if debug else "nc"
    if key not in _cache:
        _cache[key] = _build_program(debug)
    return _cache[key]


def _shard_inputs(x, xa, ln_w, ln_b, Wq, Wkv, Wout):
    x = np.asarray(x, dtype=np.float32)
    xa = np.asarray(xa, dtype=np.float32)
    ln_w = np.ascontiguousarray(np.asarray(ln_w, dtype=np.float32))
    ln_b = np.ascontiguousarray(np.asarray(ln_b, dtype=np.float32))
    Wq = np.asarray(Wq, dtype=np.float32)
    Wkv = np.asarray(Wkv, dtype=np.float32)
    Wout = np.asarray(Wout, dtype=np.float32)
    in_maps = []
    for core in range(8):
        b, g = core // 2, core % 2
        cols = slice(g * HG, (g + 1) * HG)
        in_maps.append({
            "xb": np.ascontiguousarray(x[b]),
            "xab": np.ascontiguousarray(xa[b]),
            "lnw": ln_w,
            "lnb": ln_b,
            "wq": np.ascontiguousarray(Wq[:, cols]),
            "wk": np.ascontiguousarray(Wkv[:, :D][:, cols]),
            "wv": np.ascontiguousarray(Wkv[:, D:][:, cols]),
            "wo": np.ascontiguousarray(Wout[cols, :]),
        })
    return in_maps


def kernel(x, xa, ln_w, ln_b, Wq, Wkv, Wout):
    from concourse.bass_utils import run_bass_kernel_spmd
    nc = _get_program()
    in_maps = _shard_inputs(x, xa, ln_w, ln_b, Wq, Wkv, Wout)
    res = run_bass_kernel_spmd(nc, in_maps, list(range(8)))
    out_x = np.empty((B, N, D), np.float32)
    out_xa = np.empty((B, N, D), np.float32)
    for b in range(B):
        out_x[b] = res.results[2 * b]["ox"] + res.results[2 * b + 1]["ox"]
        out_xa[b] = res.results[2 * b]["oxa"] + res.results[2 * b + 1]["oxa"]
    return out_x, out_xa
